# revision 28
# baseline (speedup 1.0000x reference)
"""Trainium2 SPMD kernel for NativeSparseAttention (B=2,S=1024,D=1024,H=16,HD=64).

Sharding: tensor-parallel over heads. 8 cores x 2 heads (128 cols of D) each.
Every core computes:
  - its head-shard of q/k/v (bf16), all three branch outputs for its heads,
  - a replicated high-precision "importance" chain (exact top-2 selection),
  - a partial out@Wo (rows of Wo owned by its heads).
Host sums the 8 partial [2048,1024] fp16 outputs and adds bo.

The importance chain (imp = x @ WqWk^T @ cx^T, top-2 per query) needs ~1e-5
relative precision to reproduce the reference's exact top-2 picks (min margin
2.1e-5). Instead of fp32 matmuls (4x slower on the PE), x and WkWq^T are
shipped as exact bf16 hi/lo pairs and each product runs as three bf16-rate
matmuls (hi*hi + hi*lo + lo*hi, fp32 PSUM accumulation); the dropped lo*lo
term is ~4e-6 relative. Verified: zero selection flips on the problem inputs.

Branch 2 (top-k block selection) is computed densely against the 1024
selectable tokens (both source batches stacked into 128-row key tiles); the
data-dependent selection enters as a one-hot multiplicative mask on
exp(scores) built with iota==top_idx compares. No gathers, no collectives.

Layouts are feature-major ("transposed", [feature, token]) so matmuls chain
without transposing activations; only V and the one-hot masks are transposed
on the PE. Each branch's attention-value matmul carries an extra ones-column
so the softmax denominator comes out as row 64 of the same PSUM tile ("aug"
trick). Gating transposes the denominators to token-partitions, takes cheap
reciprocals there, and broadcasts the per-token gate/denominator ratio back
across partitions with c=1 ones-matmuls; it runs incrementally per branch so
only branch 3's gating remains after the last branch.

Scheduling: constants and projection weights ride in two packed DMAs so the
single HBM queue reaches the x tensors quickly, in priority order (x hi, x
lo, then the WkWq pair). The top-k / one-hot chain (vector+gpsimd) overlaps
branches 1-2 on the PE.
"""

import math
from contextlib import ExitStack

import numpy as np
import ml_dtypes

B, S, D = 2, 1024, 1024
H, HD = 16, 64
CB, SB, J, W = 16, 8, 2, 256
Nc = S // CB  # 64 compressed blocks
T = B * S  # 2048 tokens total
NCORES = 8
HPC = H // NCORES  # 2 heads per core
C = HPC * HD  # 128 feature cols per core
INV = 1.0 / math.sqrt(HD)

bf16 = ml_dtypes.bfloat16

_PROG = None  # cached nc


def _build_program():
    import concourse.bass as bass
    import concourse.bacc as bacc
    import concourse.mybir as mybir
    import concourse.tile as tile
    from concourse.masks import make_identity

    dt = mybir.dt
    Alu = mybir.AluOpType
    Act = mybir.ActivationFunctionType

    nc = bacc.Bacc("TRN2", target_bir_lowering=False, debug=False,
                   num_devices=NCORES)

    # ---- DRAM parameters (per-core data supplied by the host) ----
    # packed fp32 constants: [wkc 0:16 | wvc 16:32 | wpeT 32:48 | iota 48:112
    #                         | m0 112:240 | m2 240:368 | bg col 368]
    cpack = nc.declare_dram_parameter("cpack", [128, 369], dt.float32, isOutput=False)
    # packed per-core projection weights: [wq 0:128 | wk 128:256 | wv 256:384
    #                                      | wg 384:387]
    wpack = nc.declare_dram_parameter("wpack", [D, 387], dt.bfloat16, isOutput=False)
    wo16 = nc.declare_dram_parameter("wo16", [C, D], dt.bfloat16, isOutput=False)
    xT16 = nc.declare_dram_parameter("xT16", [D, T], dt.bfloat16, isOutput=False)
    xR16 = nc.declare_dram_parameter("xR16", [D, T], dt.bfloat16, isOutput=False)
    wkwqh = nc.declare_dram_parameter("wkwqh", [D, D], dt.bfloat16, isOutput=False)
    wkwql = nc.declare_dram_parameter("wkwql", [D, D], dt.bfloat16, isOutput=False)

    out_part = nc.declare_dram_parameter("out_part", [T, D], dt.float16, isOutput=True)

    with tile.TileContext(nc) as tc, ExitStack() as ctx:
        sync = nc.sync

        def pool(name, bufs=1, space="SBUF"):
            return ctx.enter_context(tc.tile_pool(name=name, bufs=bufs, space=space))

        def arr(p, n, shape, dtype, tag):
            return [p.tile(shape, dtype, tag=f"{tag}{i}", name=f"{tag}{i}")
                    for i in range(n)]

        # ---------- constants (one DMA) ----------
        cpool = pool("consts")
        cpk = cpool.tile([128, 369], dt.float32, tag="cpk", name="cpk")
        sync.dma_start(cpk[:], cpack[:])
        c_wkc = cpk[:, 0:16]
        c_wvc = cpk[:, 16:32]
        c_wpeT = cpk[:, 32:48]
        c_iota = cpk[:, 48:112]
        c_m0 = cpk[:, 112:240]
        c_m2 = cpk[:, 240:368]
        c_bg = cpk[0:3, 368:369]
        c_ident = cpool.tile([128, 128], dt.bfloat16, tag="ident", name="ident")
        c_id32 = cpool.tile([128, 128], dt.float32, tag="id32", name="id32")
        make_identity(nc, c_ident[:])
        make_identity(nc, c_id32[:])

        # ---------- packed projection weights (8 DMAs) + Wo ----------
        wpool = pool("wpool")
        wpk = arr(wpool, 8, [128, 387], dt.bfloat16, "wpk_")
        wo2 = wpool.tile([C, D], dt.bfloat16, tag="wo2", name="wo2")
        for i in range(8):
            sync.dma_start(wpk[i][:], wpack[i * 128:(i + 1) * 128, :])
        sync.dma_start(wo2[:], wo16[:])

        # PSUM pools: psA (2 banks) + psS (4 banks) + psb2 (2 banks) = 8
        psA = pool("psA", bufs=2, space="PSUM")
        psS = pool("psS", bufs=4, space="PSUM")
        psb2 = pool("psb2", bufs=1, space="PSUM")

        def pa():
            return psA.tile([128, 512], dt.float32, tag="pa", name="pa")

        def ps(p_=128, f=512):
            t_ = psS.tile([128, 512], dt.float32, tag="ps", name="ps")
            return t_[0:p_, 0:f]

        def psT(p_=128, f=512):
            t_ = psS.tile([128, 512], dt.bfloat16, tag="ps", name="psT")
            return t_[0:p_, 0:f]

        # ---------- persistent pools ----------
        cxp = pool("cxp")
        cxkT = arr(cxp, 8, [128, 128], dt.float32, "cxk_")
        cxh = arr(cxp, 8, [128, 128], dt.bfloat16, "cxh_")
        cxl = arr(cxp, 8, [128, 128], dt.bfloat16, "cxl_")
        actp = pool("actp")
        qT = actp.tile([C, T], dt.bfloat16, tag="qT", name="qT")
        kT = actp.tile([C, T], dt.bfloat16, tag="kT", name="kT")
        vT = actp.tile([C, T], dt.bfloat16, tag="vT", name="vT")
        growb = actp.tile([3, T], dt.bfloat16, tag="growb", name="growb")
        otp = pool("otp")
        # OTS[rank]: [128 = (src n), 1024 s] bf16; masks output-batch `rank`
        OTS = [otp.tile([128, S], dt.bfloat16, tag=f"OTS{r}", name=f"OTS{r}")
               for r in range(2)]
        mpool = pool("mp")
        Mt = arr(mpool, 8, [128, 128], dt.float32, "M_")
        Mh = arr(mpool, 8, [128, 128], dt.bfloat16, "Mh_")
        Ml = arr(mpool, 8, [128, 128], dt.bfloat16, "Ml_")
        # per-(b,st) fp32 importance tiles [s, n] for the top-k chain
        impp = pool("impp")
        impt = [[impp.tile([128, Nc], dt.float32, tag=f"imp{b}{st}",
                           name=f"imp{b}{st}") for st in range(8)]
                for b in range(B)]
        # compress + V-layout tiles
        ckp = pool("ckp")
        ckTs = ckp.tile([C, 128], dt.bfloat16, tag="ckTs", name="ckTs")
        ck32 = ckp.tile([C, 128], dt.float32, tag="ck32", name="ck32")
        cvT = ckp.tile([C, 128], dt.float32, tag="cvT", name="cvT")
        cva = [ckp.tile([Nc, 130], dt.bfloat16, tag=f"cva{b}", name=f"cva{b}")
               for b in range(B)]
        v_tok = arr(pool("vtp"), 16, [128, 130], dt.bfloat16, "vtok_")
        vbp = pool("vbp")
        Vblk2 = [vbp.tile([128, 130], dt.bfloat16, tag=f"vb{t}", name=f"vb{t}")
                 for t in range(SB)]
        kSel = [vbp.tile([128, 128], dt.bfloat16, tag=f"ks{t}", name=f"ks{t}")
                for t in range(SB)]
        idxp = pool("idxp", bufs=1)
        ohs = [[[idxp.tile([128, Nc], dt.bfloat16, tag=f"oh{b}{st}{r}",
                           name=f"oh{b}{st}{r}") for r in range(2)]
                for st in range(8)] for b in range(B)]

        # ---------- staged big inputs ----------
        ctxA = ExitStack()  # x hi/lo — released after imp^T
        ctxB = ExitStack()  # WkWq hi/lo — released after M^T
        ctxD = ExitStack()  # fp32 x scratch — released after phase 1
        xap = ctxA.enter_context(tc.tile_pool(name="xap", bufs=1))
        xt16 = arr(xap, 8, [128, T], dt.bfloat16, "xt16_")
        xr16 = arr(xap, 8, [128, T], dt.bfloat16, "xr16_")
        wqp = ctxB.enter_context(tc.tile_pool(name="wqp", bufs=1))
        wkh = arr(wqp, 8, [128, D], dt.bfloat16, "wkh_")
        wkl = arr(wqp, 8, [128, D], dt.bfloat16, "wkl_")
        cxyp = ctxD.enter_context(tc.tile_pool(name="cxyp", bufs=2))
        # priority order on one queue: x hi (phase 2), x lo, WkWq pair
        for i in range(8):
            sync.dma_start(xt16[i][:], xT16[i * 128:(i + 1) * 128, :])
        for i in range(8):
            sync.dma_start(xr16[i][:], xR16[i * 128:(i + 1) * 128, :])
        for i in range(8):
            sync.dma_start(wkh[i][:], wkwqh[i * 128:(i + 1) * 128, :])
            sync.dma_start(wkl[i][:], wkwql[i * 128:(i + 1) * 128, :])

        # =========================================================
        # Phase 2: projections qT/kT/vT [C, T] bf16 (+pe for k,v), gates
        # =========================================================
        for ch in range(4):
            csl = slice(ch * 512, (ch + 1) * 512)
            for which, wsl, dest in (("q", slice(0, 128), qT),
                                     ("k", slice(128, 256), kT),
                                     ("v", slice(256, 384), vT)):
                pp = pa()
                for i in range(8):
                    nc.tensor.matmul(pp[:], wpk[i][:, wsl], xt16[i][:, csl],
                                     start=(i == 0), stop=(i == 7))
                if which == "q":
                    nc.scalar.copy(dest[:, csl], pp[:])
                else:
                    # add tiled positional rows: out = psum + wpeT (tok%16)
                    dv = dest[:, csl].rearrange("p (r t) -> p r t", t=CB)
                    pv = pp[:].rearrange("p (r t) -> p r t", t=CB)
                    peb = c_wpeT[:, None, :].to_broadcast((C, 512 // CB, CB))
                    nc.vector.tensor_tensor(out=dv, in0=pv, in1=peb, op=Alu.add)
            pg = ps(3, 512)
            for i in range(8):
                nc.tensor.matmul(pg[:], wpk[i][:, 384:387], xt16[i][:, csl],
                                 start=(i == 0), stop=(i == 7))
            nc.scalar.activation(growb[:, csl], pg[:], Act.Sigmoid, bias=c_bg)
        # =========================================================
        # Phase 1: reconstruct x fp32 per tile, cxkT = compress(x) fp32,
        # then split cx into bf16 hi/lo for the M^T matmuls.
        # =========================================================
        wkc_b = c_wkc[:, None, :].to_broadcast((128, 128, CB))
        for i in range(8):
            xf = cxyp.tile([128, T], dt.float32, tag="xf", name="xf")
            nc.gpsimd.tensor_tensor(out=xf[:], in0=xt16[i][:], in1=xr16[i][:],
                                    op=Alu.add)
            y = cxyp.tile([128, T], dt.float32, tag="cxy", name="cxy")
            yv = y[:].rearrange("p (bn t) -> p bn t", t=CB)
            nc.gpsimd.tensor_tensor(
                out=yv, in0=xf[:].rearrange("p (bn t) -> p bn t", t=CB),
                in1=wkc_b, op=Alu.mult)
            nc.vector.tensor_reduce(out=cxkT[i][:], in_=yv,
                                    axis=mybir.AxisListType.X, op=Alu.add)
            nc.vector.tensor_copy(cxh[i][:], cxkT[i][:])
            nc.vector.tensor_tensor(out=cxl[i][:], in0=cxkT[i][:],
                                    in1=cxh[i][:], op=Alu.subtract)

        ctxD.close()

        # compress kT/vT -> ck32/cvT (gpsimd multiply + vector reduce)
        ctxE = ExitStack()
        cmpp = ctxE.enter_context(tc.tile_pool(name="cmpp", bufs=1))
        cmpy = [cmpp.tile([C, T], dt.bfloat16, tag=f"cy{j}", name=f"cy{j}")
                for j in range(2)]
        for j, (src_t, wt, tgt) in enumerate(
                ((kT, c_wkc, ck32), (vT, c_wvc, cvT))):
            yv = cmpy[j][:].rearrange("p (bn t) -> p bn t", t=CB)
            nc.gpsimd.tensor_tensor(
                out=yv, in0=src_t[:].rearrange("p (bn t) -> p bn t", t=CB),
                in1=wt[:, None, :].to_broadcast((128, 128, CB)), op=Alu.mult)
            nc.vector.tensor_reduce(out=tgt[:], in_=yv,
                                    axis=mybir.AxisListType.X, op=Alu.add)
        nc.vector.tensor_copy(ckTs[:], ck32[:])
        ctxE.close()

        def vtok_prep(tt):
            pv = psT(128, 128)
            nc.tensor.transpose(pv[:], vT[:, tt * 128:(tt + 1) * 128],
                                c_ident[:])
            dv = v_tok[tt][:].rearrange("p (h c) -> p h c", h=2)
            nc.vector.tensor_copy(
                dv[:, :, 0:64], pv[:].rearrange("p (h c) -> p h c", h=2))
            nc.gpsimd.memset(dv[:, :, 64:65], 1.0)

        # ---- V-layout prep: fills the PE while phase 1 runs on vector ----
        for tt in range(16):
            vtok_prep(tt)
        # cva (compressed V per batch) from cvT
        for b in range(B):
            pv = ps(Nc, 128)
            nc.tensor.transpose(pv[:], cvT[:, b * Nc:(b + 1) * Nc], c_id32[:])
            dv = cva[b][:].rearrange("p (h c) -> p h c", h=2)
            nc.vector.tensor_copy(
                dv[:, :, 0:64], pv[:].rearrange("p (h c) -> p h c", h=2))
            nc.gpsimd.memset(dv[:, :, 64:65], 1.0)

        # Vblk2[t]: [128 = (src n), 130] bf16 — selectable tokens src*1024+n*16+t
        # kSel[t]:  [128 c, 128 = (src n)] bf16 — their keys, feature-major
        vTv = vT[:].rearrange("p (b n t) -> p b t n", b=B, t=CB)
        kTv = kT[:].rearrange("p (b n t) -> p b t n", b=B, t=CB)
        for t in range(SB):
            for src in range(2):
                pvf = psS.tile([128, 512], dt.bfloat16, tag="ps", name="psT")
                pv = pvf[src * Nc:(src + 1) * Nc, 0:128]
                nc.tensor.transpose(pv, vTv[:, src, t, :], c_ident[:])
                dv = Vblk2[t][src * Nc:(src + 1) * Nc, :].rearrange(
                    "p (h c) -> p h c", h=2)
                nc.vector.tensor_copy(
                    dv[:, :, 0:64], pv.rearrange("p (h c) -> p h c", h=2))
                nc.gpsimd.memset(dv[:, :, 64:65], 1.0)
                nc.vector.tensor_copy(
                    kSel[t][:, src * Nc:(src + 1) * Nc], kTv[:, src, t, :])


        # =========================================================
        # Phase 3a: M^T [bn, e] then M; imp^T [bn, s] then impt [s, n].
        # All products are 3-term bf16 hi/lo splits at full PE rate.
        # =========================================================
        mtp = ctxB.enter_context(tc.tile_pool(name="mtp", bufs=2))
        for ech in range(2):
            esl = slice(ech * 512, (ech + 1) * 512)
            pmt = ps(128, 512)
            n3 = 0
            for i in range(8):
                for lh, rh in ((cxh[i], wkh[i]), (cxh[i], wkl[i]),
                               (cxl[i], wkh[i])):
                    nc.tensor.matmul(pmt[:], lh[:], rh[:, esl],
                                     start=(n3 == 0), stop=(n3 == 23))
                    n3 += 1
            mts = mtp.tile([128, 512], dt.float32, tag="mts", name="mts")
            nc.vector.tensor_copy(mts[:], pmt[:])
            for j in range(4):
                e = ech * 4 + j
                pt = ps(128, 128)
                nc.tensor.transpose(pt[:], mts[:, j * 128:(j + 1) * 128],
                                    c_id32[:])
                nc.scalar.copy(Mt[e][:], pt[:])
                nc.vector.tensor_copy(Mh[e][:], Mt[e][:])
                nc.vector.tensor_tensor(out=Ml[e][:], in0=Mt[e][:],
                                        in1=Mh[e][:], op=Alu.subtract)
        ctxB.close()

        itp = ctxA.enter_context(tc.tile_pool(name="itp", bufs=2))
        for b in range(B):
            for sch in range(2):
                ssl = slice(b * S + sch * 512, b * S + (sch + 1) * 512)
                pimp = ps(128, 512)
                n3 = 0
                for e in range(8):
                    for lh, rh in ((Mh[e], xt16[e]), (Mh[e], xr16[e]),
                                   (Ml[e], xt16[e])):
                        nc.tensor.matmul(pimp[:], lh[:], rh[:, ssl],
                                         start=(n3 == 0), stop=(n3 == 23))
                        n3 += 1
                impTs = itp.tile([128, 512], dt.float32, tag="impTs",
                                 name="impTs")
                nc.vector.tensor_copy(impTs[:], pimp[:])
                for j in range(4):
                    st = sch * 4 + j
                    pt = ps(128, 128)
                    nc.tensor.transpose(pt[:], impTs[:, j * 128:(j + 1) * 128],
                                        c_id32[:])
                    nc.scalar.copy(impt[b][st][:], pt[:, b * Nc:(b + 1) * Nc])
                # top-2 + one-hot for this chunk's four st's (vector + gpsimd)
                for j in range(4):
                    st = sch * 4 + j
                    mx8 = idxp.tile([128, 8], dt.float32, tag="mx8", name="mx8")
                    ix8 = idxp.tile([128, 8], dt.uint32, tag="ix8", name="ix8")
                    nc.vector.max(out=mx8[:], in_=impt[b][st][:])
                    nc.vector.max_index(out=ix8[:], in_max=mx8[:],
                                        in_values=impt[b][st][:])
                    ixf = idxp.tile([128, 2], dt.float32, tag="ixf", name="ixf")
                    nc.vector.tensor_copy(ixf[:], ix8[:, 0:2])
                    for r in range(2):
                        nc.gpsimd.tensor_scalar(
                            out=ohs[b][st][r][:], in0=c_iota,
                            scalar1=ixf[:, r:r + 1], scalar2=None,
                            op0=Alu.is_equal)
        ctxA.close()

        # =========================================================
        # Branches. ofull[k][b][h]: [65, S] bf16
        #   rows 0-63 = numerator values, row 64 = softmax denominator.
        # Gating runs incrementally: k=0,1 after branch 2, k=2 in the tail.
        # =========================================================
        brp = pool("brp")
        ofull = [[[brp.tile([HD + 1, S], dt.bfloat16, tag=f"o{k}_{b}{h}",
                            name=f"o{k}_{b}{h}")
                   for h in range(2)] for b in range(2)] for k in range(3)]
        exp1 = pool("exp1", bufs=1)
        expp = pool("expp", bufs=3)
        gp = pool("gp", bufs=2)
        gatedT = gp.tile([C, T], dt.bfloat16, tag="gatedT", name="gatedT")
        c_ones1 = gp.tile([1, HD], dt.bfloat16, tag="ones1", name="ones1")
        nc.gpsimd.memset(c_ones1[:], 1.0)
        d7 = [gp.tile([7, S], dt.bfloat16, tag=f"d7{b}", name=f"d7{b}")
              for b in range(B)]
        d3 = [gp.tile([3, S], dt.bfloat16, tag=f"d3{b}", name=f"d3{b}")
              for b in range(B)]
        a4 = [gp.tile([4, S], dt.bfloat16, tag=f"a4{b}", name=f"a4{b}")
              for b in range(B)]
        a2 = [gp.tile([2, S], dt.bfloat16, tag=f"a2{b}", name=f"a2{b}")
              for b in range(B)]
        # alpha rows as separate [1, S] tiles so matmul base partition is 0
        ar = [[gp.tile([1, S], dt.bfloat16, tag=f"ar{b}{r}", name=f"ar{b}{r}")
               for r in range(6)] for b in range(B)]
        # h=1 gating accumulators (DMA-shifted into gatedT after k=2)
        gacc = [[gp.tile([HD, 512], dt.bfloat16, tag=f"gacc{b}{ch}",
                         name=f"gacc{b}{ch}") for ch in range(2)]
                for b in range(B)]
        rcp = pool("rcp", bufs=3)
        fop = pool("fop", bufs=2)

        def gate_round(b, ks):
            """Apply gating for branches `ks` (list of k) of batch b."""
            for h in range(HPC):
                for ch in range(2):
                    csl = slice(ch * 512, (ch + 1) * 512)
                    tsl = slice(b * S + ch * 512, b * S + (ch + 1) * 512)
                    dst = gacc[b][ch][:] if h == 1 else gatedT[0:HD, tsl]
                    for k in ks:
                        row = k * 2 + h
                        pr = ps(HD, 512)
                        nc.tensor.matmul(pr[:], c_ones1[:],
                                         ar[b][row][:, csl],
                                         start=True, stop=True)
                        ob = ofull[k][b][h][0:HD, csl]
                        if k == 0:
                            nc.vector.tensor_tensor(out=dst, in0=ob, in1=pr[:],
                                                    op=Alu.mult)
                        else:
                            tmp = rcp.tile([HD, 512], dt.bfloat16, tag="gtmp",
                                           name="gtmp")
                            nc.vector.tensor_tensor(out=tmp[:], in0=ob,
                                                    in1=pr[:], op=Alu.mult)
                            nc.vector.tensor_tensor(out=dst, in0=dst,
                                                    in1=tmp[:], op=Alu.add)

        # ---- branch 1: compressed attention ----
        for b in range(B):
            for h in range(HPC):
                hsl = slice(h * HD, (h + 1) * HD)
                asl = slice(h * 65, (h + 1) * 65)
                p1 = exp1.tile([Nc, S], dt.bfloat16, tag="p1t", name="p1t")
                for ch in range(2):
                    csl = slice(ch * 512, (ch + 1) * 512)
                    tsl = slice(b * S + ch * 512, b * S + (ch + 1) * 512)
                    pp = ps(Nc, 512)
                    nc.tensor.matmul(pp[:], ckTs[hsl, b * Nc:(b + 1) * Nc],
                                     qT[hsl, tsl], start=True, stop=True)
                    nc.scalar.activation(p1[:, csl], pp[:], Act.Exp, scale=INV)
                for ch in range(2):
                    csl = slice(ch * 512, (ch + 1) * 512)
                    po = ps(HD + 1, 512)
                    nc.tensor.matmul(po[:], cva[b][:, asl],
                                     p1[:, csl], start=True, stop=True)
                    nc.scalar.copy(ofull[0][b][h][:, csl], po[:])

        # ---- phase 3c: transpose one-hots into the stacked OTS masks ----
        for b in range(B):
            for st in range(8):
                for r in range(2):
                    ptf = psS.tile([128, 512], dt.bfloat16, tag="ps",
                                   name="psT")
                    pto = ptf[b * Nc:(b + 1) * Nc, 0:128]
                    nc.tensor.transpose(pto, ohs[b][st][r][:], c_ident[:])
                    nc.vector.tensor_copy(
                        OTS[r][b * Nc:(b + 1) * Nc, st * 128:(st + 1) * 128],
                        pto)

        # ---- branch 2: selected-block attention (dense + one-hot mask) ----
        for b in range(B):
            for h in range(HPC):
                hsl = slice(h * HD, (h + 1) * HD)
                asl = slice(h * 65, (h + 1) * 65)
                po2 = [psb2.tile([HD + 1, 512], dt.float32, tag=f"acc{ch}",
                                 name=f"acc{ch}") for ch in range(2)]
                for t in range(SB):
                    p2 = expp.tile([128, S], dt.bfloat16, tag="p2t", name="p2t")
                    for ch in range(2):
                        csl = slice(ch * 512, (ch + 1) * 512)
                        tsl = slice(b * S + ch * 512, b * S + (ch + 1) * 512)
                        pp = ps(128, 512)
                        nc.tensor.matmul(pp[:], kSel[t][hsl, :],
                                         qT[hsl, tsl], start=True, stop=True)
                        nc.scalar.activation(p2[:, csl], pp[:], Act.Exp,
                                             scale=INV)
                    # mask with one-hot of the selected blocks (both srcs)
                    nc.vector.tensor_tensor(out=p2[:], in0=p2[:],
                                            in1=OTS[b][:], op=Alu.mult)
                    for ch in range(2):
                        csl = slice(ch * 512, (ch + 1) * 512)
                        nc.tensor.matmul(
                            po2[ch][:], Vblk2[t][:, asl], p2[:, csl],
                            start=(t == 0), stop=(t == SB - 1))
                for ch in range(2):
                    csl = slice(ch * 512, (ch + 1) * 512)
                    nc.scalar.copy(ofull[1][b][h][:, csl], po2[ch][:])

        # ---- gating for k=0,1 (branches 1 and 2) ----
        for b in range(B):
            nc.vector.tensor_copy(d7[b][0:3, :], growb[:, b * S:(b + 1) * S])
            for k in range(2):
                for h in range(HPC):
                    sync.dma_start(d7[b][3 + k * 2 + h:4 + k * 2 + h, :],
                                   ofull[k][b][h][HD:HD + 1, :])
        for b in range(B):
            for st in range(8):
                ssl = slice(st * 128, (st + 1) * 128)
                pt = psT(128, 7)
                nc.tensor.transpose(pt[:], d7[b][:, ssl], c_ident[0:7, 0:7])
                rc = rcp.tile([128, 4], dt.float32, tag="rc4", name="rc4")
                nc.vector.reciprocal(rc[:], pt[:, 3:7])
                at = rcp.tile([128, 4], dt.bfloat16, tag="at4", name="at4")
                nc.vector.tensor_tensor(
                    out=at[:].rearrange("p (k h) -> p k h", k=2),
                    in0=rc[:].rearrange("p (k h) -> p k h", k=2),
                    in1=pt[:, 0:2][:, :, None].to_broadcast((128, 2, 2)),
                    op=Alu.mult)
                pb = psT(4, 128)
                nc.tensor.transpose(pb[:], at[:], c_ident[:])
                nc.vector.tensor_copy(a4[b][:, ssl], pb[:])
            for r in range(4):
                sync.dma_start(ar[b][r][:], a4[b][r:r + 1, :])
        gate_round(0, [0, 1])
        gate_round(1, [0, 1])

        # ---- branch 3: causal sliding window (strip-tiled scores) ----
        # rolling window: produce exp-strip kt, then immediately accumulate
        # the value matmuls for query-chunk st=kt from strips kt-2..kt
        strp = pool("strp", bufs=4)
        for b in range(B):
            for h in range(HPC):
                hsl = slice(h * HD, (h + 1) * HD)
                asl = slice(h * 65, (h + 1) * 65)
                strips = {}
                po4 = None
                for kt in range(8):
                    nst = min(3, 8 - kt)
                    strip = nst * 128
                    ktsl = slice(b * S + kt * 128, b * S + (kt + 1) * 128)
                    qsl = slice(b * S + kt * 128, b * S + kt * 128 + strip)
                    pp = ps(128, 512)
                    nc.tensor.matmul(pp[:, 0:strip], kT[hsl, ktsl],
                                     qT[hsl, qsl], start=True, stop=True)
                    # diag (st==kt): causal mask; st==kt+2: window mask
                    nc.vector.tensor_tensor(out=pp[:, 0:128], in0=pp[:, 0:128],
                                            in1=c_m0, op=Alu.add)
                    if nst == 3:
                        nc.vector.tensor_tensor(out=pp[:, 256:384],
                                                in0=pp[:, 256:384],
                                                in1=c_m2, op=Alu.add)
                    p3t = strp.tile([128, 512], dt.bfloat16, tag="p3s",
                                    name="p3s")
                    nc.scalar.activation(p3t[:, 0:strip], pp[:, 0:strip],
                                         Act.Exp, scale=INV)
                    strips[kt] = p3t
                    st = kt
                    si = st % 4
                    if si == 0:
                        po4 = psb2.tile([HD + 1, 512], dt.float32,
                                        tag=f"acc{(st // 4) % 2}", name="po4")
                    kts = [k2_ for k2_ in (st - 2, st - 1, st) if k2_ >= 0]
                    for ki, ktv in enumerate(kts):
                        nc.tensor.matmul(
                            po4[:, si * 128:(si + 1) * 128],
                            v_tok[b * 8 + ktv][:, asl],
                            strips[ktv][:, (st - ktv) * 128:(st - ktv + 1) * 128],
                            start=(ki == 0), stop=(ki == len(kts) - 1),
                            skip_group_check=True)
                    if si == 3:
                        nc.scalar.copy(
                            ofull[2][b][h][:, (st - 3) * 128:(st + 1) * 128],
                            po4[:])
                    strips.pop(kt - 2, None)

        # ---- gating for k=2 (branch 3) + output projection, per batch ----
        for b in range(B):
            sync.dma_start(d3[b][0:1, :], growb[2:3, b * S:(b + 1) * S])
            for h in range(HPC):
                sync.dma_start(d3[b][1 + h:2 + h, :],
                               ofull[2][b][h][HD:HD + 1, :])
        for b in range(B):
            for st in range(8):
                ssl = slice(st * 128, (st + 1) * 128)
                pt = psT(128, 3)
                nc.tensor.transpose(pt[:], d3[b][:, ssl], c_ident[0:3, 0:3])
                rc = rcp.tile([128, 2], dt.float32, tag="rc2", name="rc2")
                nc.vector.reciprocal(rc[:], pt[:, 1:3])
                at = rcp.tile([128, 2], dt.bfloat16, tag="at2", name="at2")
                nc.vector.tensor_tensor(
                    out=at[:].rearrange("p (k h) -> p k h", k=1),
                    in0=rc[:].rearrange("p (k h) -> p k h", k=1),
                    in1=pt[:, 0:1][:, :, None].to_broadcast((128, 1, 2)),
                    op=Alu.mult)
                pb = psT(2, 128)
                nc.tensor.transpose(pb[:], at[:], c_ident[:])
                nc.vector.tensor_copy(a2[b][:, ssl], pb[:])
            for r in range(2):
                sync.dma_start(ar[b][4 + r][:], a2[b][r:r + 1, :])
        for b in range(B):
            gate_round(b, [2])
            for ch in range(2):
                tsl = slice(b * S + ch * 512, b * S + (ch + 1) * 512)
                sync.dma_start(gatedT[HD:C, tsl], gacc[b][ch][:])
            # output projection for this batch (fp16 out, bias added on host)
            for st in range(b * 8, (b + 1) * 8):
                ssl = slice(st * 128, (st + 1) * 128)
                ot = fop.tile([128, D], dt.float16, tag="fo", name="fo")
                for ch in range(2):
                    csl = slice(ch * 512, (ch + 1) * 512)
                    pf = pa()
                    nc.tensor.matmul(pf[:], gatedT[:, ssl], wo2[:, csl],
                                     start=True, stop=True)
                    nc.scalar.copy(ot[:, csl], pf[:])
                sync.dma_start(out_part[ssl, :], ot[:])

    nc.compile()
    return nc


def _prep_inputs(inputs):
    """Build the 8 per-core input maps from the full problem inputs."""
    x = np.ascontiguousarray(np.asarray(inputs["x"], dtype=np.float32))
    Wq = np.asarray(inputs["Wq"], dtype=np.float32)
    Wk = np.asarray(inputs["Wk"], dtype=np.float32)
    Wv = np.asarray(inputs["Wv"], dtype=np.float32)
    Wo = np.asarray(inputs["Wo"], dtype=np.float32)
    Wg = np.asarray(inputs["Wg"], dtype=np.float32)
    bg = np.asarray(inputs["bg"], dtype=np.float32)
    wkc = np.asarray(inputs["wkc"], dtype=np.float32)
    wvc = np.asarray(inputs["wvc"], dtype=np.float32)
    wpe = np.asarray(inputs["wpe"], dtype=np.float32)

    xT = np.ascontiguousarray(x.reshape(T, D).T)          # [D, T] f32
    xT16 = xT.astype(bf16)
    xR16 = (xT - xT16.astype(np.float32)).astype(bf16)    # exact lo residual
    wkwq = Wk @ Wq.T                                      # [D, D] f32
    wkwqh = wkwq.astype(bf16)
    wkwql = (wkwq - wkwqh.astype(np.float32)).astype(bf16)

    iota = np.tile(np.arange(Nc, dtype=np.float32), (128, 1))
    ii = np.arange(128)[:, None]
    jj = np.arange(128)[None, :]
    m0 = np.where(jj >= ii, 0.0, -1e6).astype(np.float32)  # delta=0 keep col>=p
    m2 = np.where(jj <= ii, 0.0, -1e6).astype(np.float32)  # delta=2 keep col<=p
    cpk = np.zeros((128, 369), np.float32)
    cpk[:, 0:16] = np.tile(wkc, (128, 1))
    cpk[:, 16:32] = np.tile(wvc, (128, 1))
    cpk[:, 48:112] = iota
    cpk[:, 112:240] = m0
    cpk[:, 240:368] = m2
    cpk[0:3, 368] = bg

    in_maps = []
    for i in range(NCORES):
        csl = slice(i * C, (i + 1) * C)
        cp = cpk.copy()
        cp[:, 32:48] = wpe.T[csl, :]
        wp = np.zeros((D, 387), bf16)
        wp[:, 0:128] = Wq[:, csl].astype(bf16)
        wp[:, 128:256] = Wk[:, csl].astype(bf16)
        wp[:, 256:384] = Wv[:, csl].astype(bf16)
        wp[:, 384:387] = Wg.astype(bf16)
        m = {
            "cpack": np.ascontiguousarray(cp),
            "wpack": np.ascontiguousarray(wp),
            "wo16": np.ascontiguousarray(Wo[csl, :]).astype(bf16),
            "xT16": xT16,
            "xR16": xR16,
            "wkwqh": wkwqh,
            "wkwql": wkwql,
        }
        in_maps.append(m)
    return in_maps


_LAST_RESULTS = None


def kernel(**inputs) -> np.ndarray:
    global _PROG, _LAST_RESULTS
    import os
    from concourse.bass_utils import run_bass_kernel_spmd

    if _PROG is None:
        _PROG = _build_program()
    nc = _PROG

    in_maps = _prep_inputs(inputs)
    trace = bool(int(os.environ.get("KERNEL_TRACE", "0")))
    res = run_bass_kernel_spmd(nc, in_maps, core_ids=list(range(NCORES)),
                               trace=trace)
    _LAST_RESULTS = res
    total = np.zeros((T, D), np.float32)
    for i in range(NCORES):
        total += res.results[i]["out_part"].astype(np.float32)
    total += np.asarray(inputs["bo"], dtype=np.float32)[None, :]
    return total.reshape(B, S, D)
